# revision 11
# baseline (speedup 1.0000x reference)
"""Trainium2 Bass kernel for nn_EncodingLayer (B=4,S=2048,D=512,H=8,DFF=2048).

Sharding: 8 cores = (batch b, query-half) pairs. Core c handles batch c//2 and
query rows [(c%2)*1024, (c%2)*1024+1024). Each core recomputes K,V for its full
batch (cheap) so no collectives are needed.

Layout strategy: everything feature-major ("transposed") on chip so all matmuls
are plain f32r (full-rate fp32) with no transpose-mode instructions.
  - x arrives host-transposed (and token-permuted so the core's q-block is
    always columns 0:1024 -> one static kernel for all cores).
  - scores computed as s^T [k, q]; exp on ScalarE; softmax denominator comes
    free from a ones-column appended to V in the attn@V matmul; normalization
    via a K=1 broadcast matmul + VectorE multiply.
  - atn is written transposed per-core ([h, k, q]); the host fixes the layout
    while gathering shards (it is a layout choice of the sharding, values are
    fully computed on device).
"""

from contextlib import ExitStack

import numpy as np

import concourse.bass as bass
import concourse.tile as tile
from concourse import bacc, mybir
from concourse.bass import ts
from concourse.bass_utils import run_bass_kernel_spmd
from concourse.masks import make_identity

F32 = mybir.dt.float32
F32R = mybir.dt.float32r
AF = mybir.ActivationFunctionType
OP = mybir.AluOpType

B, S, D, H, DFF = 4, 2048, 512, 8, 2048
HD = D // H            # 64
QB = S // 2            # 1024 q rows per core
NKT = S // 128         # 16 key tiles
NDC = D // 128         # 4 contraction chunks over D
NFT = DFF // 128       # 16 dff tiles
LN_EPS = 1e-5

_CACHED_NC = None


def _build():
    nc = bacc.Bacc()

    xT_d = nc.dram_tensor("xT", (D, S), F32R, kind="ExternalInput")
    Wq_d = nc.dram_tensor("Wq", (D, D), F32R, kind="ExternalInput")
    Wk_d = nc.dram_tensor("Wk", (D, D), F32R, kind="ExternalInput")
    Wv_d = nc.dram_tensor("Wv", (D, D), F32R, kind="ExternalInput")
    bq_d = nc.dram_tensor("bqr", (128, NDC), F32, kind="ExternalInput")
    bk_d = nc.dram_tensor("bkr", (128, NDC), F32, kind="ExternalInput")
    bv_d = nc.dram_tensor("bv", (D,), F32, kind="ExternalInput")
    W1_d = nc.dram_tensor("W1", (D, DFF), F32R, kind="ExternalInput")
    b1_d = nc.dram_tensor("b1r", (128, NFT), F32, kind="ExternalInput")
    W2_d = nc.dram_tensor("W2", (DFF, D), F32R, kind="ExternalInput")
    b2_d = nc.dram_tensor("b2", (D,), F32, kind="ExternalInput")
    gamma_d = nc.dram_tensor("gamma", (D,), F32, kind="ExternalInput")
    beta_d = nc.dram_tensor("beta", (D,), F32, kind="ExternalInput")

    atnT_d = nc.dram_tensor("atnT", (H, S, QB), F32, kind="ExternalOutput")
    y_d = nc.dram_tensor("y", (QB, D), F32, kind="ExternalOutput")

    def bcast512(dram):
        # [512] dram -> [128, 512] broadcast AP (partition stride 0)
        ap = dram.ap()
        return bass.AP(tensor=ap.tensor, offset=ap.offset, ap=[[0, 128], [1, D]])

    with tile.TileContext(nc) as tc, ExitStack() as top:
        persist = tc.alloc_tile_pool(name="persist", bufs=1)
        attn_scope = top.enter_context(tc.tile_pool(name="attn_scope", bufs=1))

        # ---- persistent tiles across phases ----
        KT = attn_scope.tile([128, NDC, S], F32R)   # K^T feature-major
        QT = attn_scope.tile([128, NDC, QB], F32R)  # Q^T feature-major (q block)
        Vp = attn_scope.tile([128, NKT, H, HD + 1], F32R)  # V token-major + ones
        AOT = persist.tile([128, NDC, QB], F32R)    # attn_out^T feature-major
        ones_r = persist.tile([1, 128], F32R)
        eps_sb = persist.tile([128, 1], F32)

        ones_f = persist.tile([1, 128], F32)
        nc.gpsimd.memset(ones_f[:], 1.0)
        nc.vector.tensor_copy(ones_r[:], ones_f[:])
        nc.vector.memset(eps_sb[:], LN_EPS)

        # ---- phase A+B: load x/Wqkv, project Q,K,V ----
        with tc.tile_pool(name="ph1", bufs=1) as ph1, \
             tc.tile_pool(name="pqk", bufs=2, space=bass.MemorySpace.PSUM) as pqk, \
             tc.tile_pool(name="pv", bufs=3, space=bass.MemorySpace.PSUM) as pv:
            xT = ph1.tile([128, NDC, S], F32R)
            Wq = ph1.tile([128, NDC, D], F32R)
            Wk = ph1.tile([128, NDC, D], F32R)
            Wv = ph1.tile([128, NDC, D], F32R)
            bq = ph1.tile([128, NDC], F32)
            bk = ph1.tile([128, NDC], F32)
            bv_bc = ph1.tile([128, D], F32)

            nc.sync.dma_start(xT[:], xT_d.rearrange("(c p) t -> p c t", p=128))
            nc.sync.dma_start(Wq[:], Wq_d.rearrange("(c p) j -> p c j", p=128))
            nc.sync.dma_start(Wk[:], Wk_d.rearrange("(c p) j -> p c j", p=128))
            nc.sync.dma_start(Wv[:], Wv_d.rearrange("(c p) j -> p c j", p=128))
            nc.sync.dma_start(bq[:], bq_d[:])
            nc.sync.dma_start(bk[:], bk_d[:])
            nc.sync.dma_start(bv_bc[:], bcast512(bv_d))

            # Q^T [D, QB]: accumulate over d_in chunks
            for mt in range(NDC):
                ps_q = pqk.tile([128, QB], F32, tag="pqk")
                for n0 in range(0, QB, 512):
                    for c in range(NDC):
                        nc.tensor.matmul(
                            ps_q[:, n0:n0 + 512],
                            Wq[:, c, ts(mt, 128)],
                            xT[:, c, n0:n0 + 512],
                            start=(c == 0), stop=(c == NDC - 1),
                        )
                nc.scalar.activation(QT[:, mt, :], ps_q[:], AF.Identity,
                                     bias=bq[:, mt:mt + 1])

            # K^T [D, S]
            for mt in range(NDC):
                for nh in range(2):
                    ps_k = pqk.tile([128, QB], F32, tag="pqk")
                    for n0 in range(0, QB, 512):
                        for c in range(NDC):
                            nc.tensor.matmul(
                                ps_k[:, n0:n0 + 512],
                                Wk[:, c, ts(mt, 128)],
                                xT[:, c, nh * QB + n0:nh * QB + n0 + 512],
                                start=(c == 0), stop=(c == NDC - 1),
                            )
                    nc.scalar.activation(KT[:, mt, nh * QB:(nh + 1) * QB], ps_k[:],
                                         AF.Identity, bias=bk[:, mt:mt + 1])

            # V token-major with ones column per head: Vp[:, tt, h, 0:64]=V, [...,64]=1
            with tc.tile_pool(name="vstage", bufs=3) as vstage:
                for tt in range(NKT):
                    ps_v = pv.tile([128, D], F32, tag="pv")
                    for c in range(NDC):
                        nc.tensor.matmul(
                            ps_v[:],
                            xT[:, c, ts(tt, 128)],
                            Wv[:, c, :],
                            start=(c == 0), stop=(c == NDC - 1),
                        )
                    vstg = vstage.tile([128, H, HD + 1], F32, tag="vstg")
                    nc.vector.memset(vstg[:, :, HD:HD + 1], 1.0)
                    nc.vector.tensor_add(
                        vstg[:, :, 0:HD],
                        ps_v[:].rearrange("p (h e) -> p h e", e=HD),
                        bv_bc[:].rearrange("p (h e) -> p h e", e=HD),
                    )
                    nc.vector.tensor_copy(Vp[:, tt], vstg[:])

        # ---- phase C: attention per (head, q-half) ----
        with tc.tile_pool(name="pt", bufs=2 * NKT + 2) as ptp, \
             tc.tile_pool(name="atn", bufs=5) as atnp, \
             tc.tile_pool(name="bc", bufs=3) as bcp, \
             tc.tile_pool(name="small", bufs=2) as smallp, \
             tc.tile_pool(name="ps_s", bufs=4, space=bass.MemorySpace.PSUM) as ps_s, \
             tc.tile_pool(name="ps_av", bufs=2, space=bass.MemorySpace.PSUM) as ps_av, \
             tc.tile_pool(name="ps_b", bufs=2, space=bass.MemorySpace.PSUM) as ps_b:
            for h in range(H):
                hp = (h % 2) * HD
                mt = h // 2
                avs, ptss = [], []
                for qh in range(2):
                    q0 = qh * 512
                    av = ps_av.tile([HD + 1, 512], F32, tag="av")
                    pts = []
                    for kt in range(NKT):
                        s_ps = ps_s.tile([128, 512], F32, tag="s")
                        nc.tensor.matmul(
                            s_ps[:],
                            KT[hp:hp + HD, mt, ts(kt, 128)],
                            QT[hp:hp + HD, mt, q0:q0 + 512],
                            start=True, stop=True,
                        )
                        pt = ptp.tile([128, 512], F32R, tag="pt")
                        nc.scalar.activation(pt[:], s_ps[:], AF.Exp, scale=0.125)
                        nc.tensor.matmul(
                            av[:],
                            Vp[:, kt, h, :],
                            pt[:],
                            start=(kt == 0), stop=(kt == NKT - 1),
                        )
                        pts.append(pt)
                    avs.append(av)
                    ptss.append(pts)

                # one reciprocal for both q-halves
                dsrc = smallp.tile([1, 1024], F32, tag="dsrc")
                nc.scalar.activation(dsrc[0:1, 0:512], avs[0][HD:HD + 1, :],
                                     AF.Identity)
                nc.scalar.activation(dsrc[0:1, 512:1024], avs[1][HD:HD + 1, :],
                                     AF.Identity)
                recip = smallp.tile([1, 1024], F32R, tag="recip")
                with nc.allow_low_precision("f32r rounding intended for matmul"):
                    nc.vector.reciprocal(recip[:], dsrc[:])

                for qh in range(2):
                    q0 = qh * 512
                    av, pts = avs[qh], ptss[qh]
                    b_ps = ps_b.tile([128, 512], F32, tag="b")
                    nc.tensor.matmul(b_ps[:], ones_r[:],
                                     recip[0:1, qh * 512:(qh + 1) * 512],
                                     start=True, stop=True)
                    # attn_out^T normalize: copy av rows to SBUF (one PSUM
                    # operand max per DVE op), then multiply by psum_b
                    av_sb = bcp.tile([HD, 512], F32, tag="avsb")
                    nc.scalar.activation(av_sb[:], av[0:HD, :], AF.Identity)
                    nc.vector.tensor_mul(
                        AOT[hp:hp + HD, mt, q0:q0 + 512],
                        av_sb[:], b_ps[0:HD, :],
                    )
                    for kt in range(NKT):
                        a_sb = atnp.tile([128, 512], F32, tag="a")
                        nc.vector.tensor_mul(a_sb[:], pts[kt][:], b_ps[:])
                        nc.sync.dma_start(
                            atnT_d[h, ts(kt, 128), q0:q0 + 512], a_sb[:])

        top.close()  # free KT/QT/Vp before FFN phase
        # ---- phase D: FFN + residual + LayerNorm ----
        with tc.tile_pool(name="ph3", bufs=1) as ph3, \
             tc.tile_pool(name="h1p", bufs=1) as h1p, \
             tc.tile_pool(name="yp", bufs=3) as yp, \
             tc.tile_pool(name="st", bufs=6) as stp, \
             tc.tile_pool(name="ps_t", bufs=2, space=bass.MemorySpace.PSUM) as ps_t, \
             tc.tile_pool(name="ps_h", bufs=3, space=bass.MemorySpace.PSUM) as ps_h, \
             tc.tile_pool(name="ps_y", bufs=2, space=bass.MemorySpace.PSUM) as ps_y:
            W1 = ph3.tile([128, NDC, DFF], F32R)
            W2 = ph3.tile([128, NFT, D], F32R)
            b1 = ph3.tile([128, NFT], F32)
            b2_bc = ph3.tile([128, D], F32)
            g_bc = ph3.tile([128, D], F32)
            be_bc = ph3.tile([128, D], F32)
            nc.sync.dma_start(W1[:], W1_d.rearrange("(c p) j -> p c j", p=128))
            nc.sync.dma_start(W2[:], W2_d.rearrange("(c p) j -> p c j", p=128))
            nc.sync.dma_start(b1[:], b1_d[:])
            nc.sync.dma_start(b2_bc[:], bcast512(b2_d))
            nc.sync.dma_start(g_bc[:], bcast512(gamma_d))
            nc.sync.dma_start(be_bc[:], bcast512(beta_d))

            # identity blocks for feature->token transpose of attn_out
            i4r = ph3.tile([128, NDC, D], F32R)
            with tc.tile_pool(name="idtmp", bufs=1) as idp:
                idf = idp.tile([128, 128], F32)
                make_identity(nc, idf[:])
                i4f = idp.tile([128, NDC, D], F32)
                nc.gpsimd.memset(i4f[:], 0.0)
                for c in range(NDC):
                    nc.vector.tensor_copy(i4f[:, c, ts(c, 128)], idf[:])
                nc.vector.tensor_copy(i4r[:], i4f[:])

            # attn_out token-major (for the residual)
            atok = ph3.tile([128, QB // 128, D], F32)
            for qt in range(QB // 128):
                t_ps = ps_t.tile([128, D], F32, tag="t")
                for c in range(NDC):
                    nc.tensor.matmul(t_ps[:], AOT[:, c, ts(qt, 128)], i4r[:, c, :],
                                     start=(c == 0), stop=(c == NDC - 1))
                nc.scalar.activation(atok[:, qt, :], t_ps[:], AF.Identity)

            std_all = stp.tile([128, 8], F32, tag="stdall")
            rstd_all = stp.tile([128, 8], F32, tag="rstdall")
            ln_tiles = []
            for qhf in range(2):
                q0 = qhf * 512
                h1 = h1p.tile([128, NFT, 512], F32R, tag="h1")
                for ft in range(NFT):
                    h_ps = ps_h.tile([128, 512], F32, tag="h")
                    for c in range(NDC):
                        nc.tensor.matmul(
                            h_ps[:],
                            W1[:, c, ts(ft, 128)],
                            AOT[:, c, q0:q0 + 512],
                            start=(c == 0), stop=(c == NDC - 1),
                        )
                    nc.scalar.activation(h1[:, ft, :], h_ps[:], AF.Relu,
                                         bias=b1[:, ft:ft + 1])
                for ql in range(4):
                    qt = qhf * 4 + ql
                    y_ps = ps_y.tile([128, D], F32, tag="y")
                    for ft in range(NFT):
                        nc.tensor.matmul(y_ps[:], h1[:, ft, ts(ql, 128)],
                                         W2[:, ft, :],
                                         start=(ft == 0), stop=(ft == NFT - 1))
                    y_sb = yp.tile([128, D], F32, tag="ysb", bufs=9)
                    nc.vector.tensor_add(y_sb[:], y_ps[:], atok[:, qt, :])
                    nc.vector.tensor_add(y_sb[:], y_sb[:], b2_bc[:])

                    stats = stp.tile([128, 6], F32, tag="st6")
                    nc.vector.bn_stats(stats[:], y_sb[:])
                    mv = stp.tile([128, 2], F32, tag="mv", bufs=9)
                    nc.vector.bn_aggr(mv[:], stats[:])
                    nc.scalar.activation(std_all[:, qt:qt + 1], mv[:, 1:2],
                                         AF.Sqrt, bias=eps_sb[:, 0:1])
                    ln_tiles.append((qt, y_sb, mv))

            nc.vector.reciprocal(rstd_all[:], std_all[:])
            for qt, y_sb, mv in ln_tiles:
                o_sb = yp.tile([128, D], F32, tag="osb")
                nc.vector.tensor_scalar(o_sb[:], y_sb[:],
                                        scalar1=mv[:, 0:1],
                                        scalar2=rstd_all[:, qt:qt + 1],
                                        op0=OP.subtract, op1=OP.mult)
                nc.vector.tensor_mul(o_sb[:], o_sb[:], g_bc[:])
                nc.vector.tensor_add(o_sb[:], o_sb[:], be_bc[:])
                nc.sync.dma_start(y_d[ts(qt, 128), :], o_sb[:])

        persist.release()

    nc.compile()
    nc.finalize()
    return nc


def _get_nc():
    global _CACHED_NC
    if _CACHED_NC is None:
        _CACHED_NC = _build()
    return _CACHED_NC


def _prep_in_maps(inputs):
    x = np.asarray(inputs["x"], dtype=np.float32)
    shared = {
        "Wq": np.ascontiguousarray(np.asarray(inputs["Wq"], np.float32)),
        "Wk": np.ascontiguousarray(np.asarray(inputs["Wk"], np.float32)),
        "Wv": np.ascontiguousarray(np.asarray(inputs["Wv"], np.float32)),
        "bqr": np.ascontiguousarray(
            np.asarray(inputs["bq"], np.float32).reshape(NDC, 128).T),
        "bkr": np.ascontiguousarray(
            np.asarray(inputs["bk"], np.float32).reshape(NDC, 128).T),
        "bv": np.ascontiguousarray(np.asarray(inputs["bv"], np.float32)),
        "W1": np.ascontiguousarray(np.asarray(inputs["W1"], np.float32)),
        "b1r": np.ascontiguousarray(
            np.asarray(inputs["b1"], np.float32).reshape(NFT, 128).T),
        "W2": np.ascontiguousarray(np.asarray(inputs["W2"], np.float32)),
        "b2": np.ascontiguousarray(np.asarray(inputs["b2"], np.float32)),
        "gamma": np.ascontiguousarray(np.asarray(inputs["gamma"], np.float32)),
        "beta": np.ascontiguousarray(np.asarray(inputs["beta"], np.float32)),
    }
    in_maps = []
    for c in range(8):
        b, qh = c // 2, c % 2
        xb = x[b]
        xp = np.concatenate([xb[qh * QB:(qh + 1) * QB],
                             xb[(1 - qh) * QB:(2 - qh) * QB]], axis=0)
        in_maps.append({**shared, "xT": np.ascontiguousarray(xp.T)})
    return in_maps


def _assemble(results):
    out = np.empty((B, S, D), np.float32)
    atn = np.empty((B, H, S, S), np.float32)
    for c in range(8):
        b, qh = c // 2, c % 2
        q0 = qh * QB
        out[b, q0:q0 + QB] = results[c]["y"]
        a = np.swapaxes(results[c]["atnT"], 1, 2)  # [H, q, k] (k in permuted order)
        if qh == 0:
            atn[b, :, q0:q0 + QB, :] = a
        else:
            atn[b, :, q0:q0 + QB, 0:QB] = a[:, :, QB:]
            atn[b, :, q0:q0 + QB, QB:] = a[:, :, 0:QB]
    return out, atn


def run(inputs, trace=False):
    nc = _get_nc()
    in_maps = _prep_in_maps(inputs)
    res = run_bass_kernel_spmd(nc, in_maps, core_ids=list(range(8)), trace=trace)
    out, atn = _assemble(res.results)
    return (out, atn), res


def kernel(**inputs):
    (out, atn), _ = run(inputs, trace=False)
    return (out, atn)


# revision 12
# speedup vs baseline: 1.0764x; 1.0764x over previous
"""Trainium2 Bass kernel for nn_EncodingLayer (B=4,S=2048,D=512,H=8,DFF=2048).

Sharding: 8 cores = (batch b, query-half) pairs. Core c handles batch c//2 and
query rows [(c%2)*1024, (c%2)*1024+1024). Each core recomputes K,V for its full
batch (cheap) so no collectives are needed.

Layout strategy: everything feature-major ("transposed") on chip so all matmuls
are plain f32r (full-rate fp32) with no transpose-mode instructions.
  - x arrives host-transposed (and token-permuted so the core's q-block is
    always columns 0:1024 -> one static kernel for all cores).
  - scores computed as s^T [k, q]; exp on ScalarE; softmax denominator comes
    free from a ones-column appended to V in the attn@V matmul; normalization
    via a K=1 broadcast matmul + VectorE multiply.
  - atn is written transposed per-core ([h, k, q]); the host fixes the layout
    while gathering shards (it is a layout choice of the sharding, values are
    fully computed on device).
"""

from contextlib import ExitStack

import numpy as np

import concourse.bass as bass
import concourse.tile as tile
from concourse import bacc, mybir
from concourse.bass import ts
from concourse.bass_utils import run_bass_kernel_spmd
from concourse.masks import make_identity

F32 = mybir.dt.float32
F32R = mybir.dt.float32r
AF = mybir.ActivationFunctionType
OP = mybir.AluOpType

B, S, D, H, DFF = 4, 2048, 512, 8, 2048
HD = D // H            # 64
QB = S // 2            # 1024 q rows per core
NKT = S // 128         # 16 key tiles
NDC = D // 128         # 4 contraction chunks over D
NFT = DFF // 128       # 16 dff tiles
LN_EPS = 1e-5

_CACHED_NC = None


def _build():
    nc = bacc.Bacc()

    xT_d = nc.dram_tensor("xT", (D, S), F32R, kind="ExternalInput")
    Wq_d = nc.dram_tensor("Wq", (D, D), F32R, kind="ExternalInput")
    Wk_d = nc.dram_tensor("Wk", (D, D), F32R, kind="ExternalInput")
    Wv_d = nc.dram_tensor("Wv", (D, D), F32R, kind="ExternalInput")
    bq_d = nc.dram_tensor("bqr", (128, NDC), F32, kind="ExternalInput")
    bk_d = nc.dram_tensor("bkr", (128, NDC), F32, kind="ExternalInput")
    bv_d = nc.dram_tensor("bv", (D,), F32, kind="ExternalInput")
    W1_d = nc.dram_tensor("W1", (D, DFF), F32R, kind="ExternalInput")
    b1_d = nc.dram_tensor("b1r", (128, NFT), F32, kind="ExternalInput")
    W2_d = nc.dram_tensor("W2", (DFF, D), F32R, kind="ExternalInput")
    b2_d = nc.dram_tensor("b2", (D,), F32, kind="ExternalInput")
    gamma_d = nc.dram_tensor("gamma", (D,), F32, kind="ExternalInput")
    beta_d = nc.dram_tensor("beta", (D,), F32, kind="ExternalInput")

    atnT_d = nc.dram_tensor("atnT", (H, S, QB), F32, kind="ExternalOutput")
    y_d = nc.dram_tensor("y", (QB, D), F32, kind="ExternalOutput")

    def bcast512(dram):
        # [512] dram -> [128, 512] broadcast AP (partition stride 0)
        ap = dram.ap()
        return bass.AP(tensor=ap.tensor, offset=ap.offset, ap=[[0, 128], [1, D]])

    with tile.TileContext(nc) as tc, ExitStack() as top:
        persist = tc.alloc_tile_pool(name="persist", bufs=1)
        attn_scope = top.enter_context(tc.tile_pool(name="attn_scope", bufs=1))

        # ---- persistent tiles across phases ----
        KT = attn_scope.tile([128, NDC, S], F32R)   # K^T feature-major
        QT = attn_scope.tile([128, NDC, QB], F32R)  # Q^T feature-major (q block)
        Vp = attn_scope.tile([128, NKT, H, HD + 1], F32R)  # V token-major + ones
        AOT = persist.tile([128, NDC, QB], F32R)    # attn_out^T feature-major
        ones_r = persist.tile([1, 128], F32R)
        eps_sb = persist.tile([128, 1], F32)

        ones_f = persist.tile([1, 128], F32)
        nc.gpsimd.memset(ones_f[:], 1.0)
        nc.vector.tensor_copy(ones_r[:], ones_f[:])
        nc.vector.memset(eps_sb[:], LN_EPS)

        # ---- phase A+B: load x/Wqkv, project Q,K,V ----
        with tc.tile_pool(name="ph1", bufs=1) as ph1, \
             tc.tile_pool(name="pqk", bufs=2, space=bass.MemorySpace.PSUM) as pqk, \
             tc.tile_pool(name="pv", bufs=3, space=bass.MemorySpace.PSUM) as pv:
            xT = ph1.tile([128, NDC, S], F32R)
            Wq = ph1.tile([128, NDC, D], F32R)
            Wk = ph1.tile([128, NDC, D], F32R)
            Wv = ph1.tile([128, NDC, D], F32R)
            bq = ph1.tile([128, NDC], F32)
            bk = ph1.tile([128, NDC], F32)
            bv_bc = ph1.tile([128, D], F32)

            nc.sync.dma_start(xT[:], xT_d.rearrange("(c p) t -> p c t", p=128))
            nc.sync.dma_start(Wq[:], Wq_d.rearrange("(c p) j -> p c j", p=128))
            nc.sync.dma_start(Wk[:], Wk_d.rearrange("(c p) j -> p c j", p=128))
            nc.sync.dma_start(Wv[:], Wv_d.rearrange("(c p) j -> p c j", p=128))
            nc.sync.dma_start(bq[:], bq_d[:])
            nc.sync.dma_start(bk[:], bk_d[:])
            nc.sync.dma_start(bv_bc[:], bcast512(bv_d))

            # Q^T [D, QB]: accumulate over d_in chunks
            for mt in range(NDC):
                ps_q = pqk.tile([128, QB], F32, tag="pqk")
                for n0 in range(0, QB, 512):
                    for c in range(NDC):
                        nc.tensor.matmul(
                            ps_q[:, n0:n0 + 512],
                            Wq[:, c, ts(mt, 128)],
                            xT[:, c, n0:n0 + 512],
                            start=(c == 0), stop=(c == NDC - 1),
                        )
                nc.scalar.activation(QT[:, mt, :], ps_q[:], AF.Identity,
                                     bias=bq[:, mt:mt + 1])

            # K^T [D, S]
            for mt in range(NDC):
                for nh in range(2):
                    ps_k = pqk.tile([128, QB], F32, tag="pqk")
                    for n0 in range(0, QB, 512):
                        for c in range(NDC):
                            nc.tensor.matmul(
                                ps_k[:, n0:n0 + 512],
                                Wk[:, c, ts(mt, 128)],
                                xT[:, c, nh * QB + n0:nh * QB + n0 + 512],
                                start=(c == 0), stop=(c == NDC - 1),
                            )
                    nc.scalar.activation(KT[:, mt, nh * QB:(nh + 1) * QB], ps_k[:],
                                         AF.Identity, bias=bk[:, mt:mt + 1])

            # V token-major with ones column per head: Vp[:, tt, h, 0:64]=V, [...,64]=1
            with tc.tile_pool(name="vstage", bufs=3) as vstage:
                for tt in range(NKT):
                    ps_v = pv.tile([128, D], F32, tag="pv")
                    for c in range(NDC):
                        nc.tensor.matmul(
                            ps_v[:],
                            xT[:, c, ts(tt, 128)],
                            Wv[:, c, :],
                            start=(c == 0), stop=(c == NDC - 1),
                        )
                    vstg = vstage.tile([128, H, HD + 1], F32, tag="vstg")
                    nc.vector.memset(vstg[:, :, HD:HD + 1], 1.0)
                    nc.vector.tensor_add(
                        vstg[:, :, 0:HD],
                        ps_v[:].rearrange("p (h e) -> p h e", e=HD),
                        bv_bc[:].rearrange("p (h e) -> p h e", e=HD),
                    )
                    nc.vector.tensor_copy(Vp[:, tt], vstg[:])

        # ---- phase C: attention per (head, q-half) ----
        with tc.tile_pool(name="pt", bufs=NKT + 6) as ptp, \
             tc.tile_pool(name="atn", bufs=6) as atnp, \
             tc.tile_pool(name="bc", bufs=3) as bcp, \
             tc.tile_pool(name="small", bufs=2) as smallp, \
             tc.tile_pool(name="ps_s", bufs=4, space=bass.MemorySpace.PSUM) as ps_s, \
             tc.tile_pool(name="ps_av", bufs=2, space=bass.MemorySpace.PSUM) as ps_av, \
             tc.tile_pool(name="ps_b", bufs=2, space=bass.MemorySpace.PSUM) as ps_b:
            for h in range(H):
                hp = (h % 2) * HD
                mt = h // 2
                for qh in range(2):
                    q0 = qh * 512
                    av = ps_av.tile([HD + 1, 512], F32, tag="av")
                    pts = []
                    for kt in range(NKT):
                        s_ps = ps_s.tile([128, 512], F32, tag="s")
                        nc.tensor.matmul(
                            s_ps[:],
                            KT[hp:hp + HD, mt, ts(kt, 128)],
                            QT[hp:hp + HD, mt, q0:q0 + 512],
                            start=True, stop=True,
                        )
                        pt = ptp.tile([128, 512], F32R, tag="pt")
                        nc.scalar.activation(pt[:], s_ps[:], AF.Exp, scale=0.125)
                        nc.tensor.matmul(
                            av[:],
                            Vp[:, kt, h, :],
                            pt[:],
                            start=(kt == 0), stop=(kt == NKT - 1),
                        )
                        pts.append(pt)

                    dsrc = smallp.tile([1, 512], F32, tag="dsrc")
                    nc.scalar.activation(dsrc[:], av[HD:HD + 1, :], AF.Identity)
                    recip = smallp.tile([1, 512], F32R, tag="recip")
                    with nc.allow_low_precision("f32r rounding for matmul"):
                        nc.vector.reciprocal(recip[:], dsrc[:])
                    b_ps = ps_b.tile([128, 512], F32, tag="b")
                    nc.tensor.matmul(b_ps[:], ones_r[:], recip[:],
                                     start=True, stop=True)
                    # attn_out^T normalize (one PSUM operand per DVE op)
                    av_sb = bcp.tile([HD, 512], F32, tag="avsb")
                    nc.scalar.activation(av_sb[:], av[0:HD, :], AF.Identity)
                    nc.vector.tensor_mul(
                        AOT[hp:hp + HD, mt, q0:q0 + 512],
                        av_sb[:], b_ps[0:HD, :],
                    )
                    for kt in range(NKT):
                        a_sb = atnp.tile([128, 512], F32, tag="a")
                        nc.vector.tensor_mul(a_sb[:], pts[kt][:], b_ps[:])
                        nc.sync.dma_start(
                            atnT_d[h, ts(kt, 128), q0:q0 + 512], a_sb[:])

        top.close()  # free KT/QT/Vp before FFN phase
        # ---- phase D: FFN + residual + LayerNorm ----
        with tc.tile_pool(name="ph3", bufs=1) as ph3, \
             tc.tile_pool(name="h1p", bufs=1) as h1p, \
             tc.tile_pool(name="yp", bufs=3) as yp, \
             tc.tile_pool(name="st", bufs=6) as stp, \
             tc.tile_pool(name="ps_t", bufs=2, space=bass.MemorySpace.PSUM) as ps_t, \
             tc.tile_pool(name="ps_h", bufs=3, space=bass.MemorySpace.PSUM) as ps_h, \
             tc.tile_pool(name="ps_y", bufs=2, space=bass.MemorySpace.PSUM) as ps_y:
            W1 = ph3.tile([128, NDC, DFF], F32R)
            W2 = ph3.tile([128, NFT, D], F32R)
            b1 = ph3.tile([128, NFT], F32)
            b2_bc = ph3.tile([128, D], F32)
            g_bc = ph3.tile([128, D], F32)
            be_bc = ph3.tile([128, D], F32)
            nc.sync.dma_start(W1[:], W1_d.rearrange("(c p) j -> p c j", p=128))
            nc.sync.dma_start(W2[:], W2_d.rearrange("(c p) j -> p c j", p=128))
            nc.sync.dma_start(b1[:], b1_d[:])
            nc.sync.dma_start(b2_bc[:], bcast512(b2_d))
            nc.sync.dma_start(g_bc[:], bcast512(gamma_d))
            nc.sync.dma_start(be_bc[:], bcast512(beta_d))

            # identity blocks for feature->token transpose of attn_out
            i4r = ph3.tile([128, NDC, D], F32R)
            with tc.tile_pool(name="idtmp", bufs=1) as idp:
                idf = idp.tile([128, 128], F32)
                make_identity(nc, idf[:])
                i4f = idp.tile([128, NDC, D], F32)
                nc.gpsimd.memset(i4f[:], 0.0)
                for c in range(NDC):
                    nc.vector.tensor_copy(i4f[:, c, ts(c, 128)], idf[:])
                nc.vector.tensor_copy(i4r[:], i4f[:])

            # attn_out token-major (for the residual)
            atok = ph3.tile([128, QB // 128, D], F32)
            for qt in range(QB // 128):
                t_ps = ps_t.tile([128, D], F32, tag="t")
                for c in range(NDC):
                    nc.tensor.matmul(t_ps[:], AOT[:, c, ts(qt, 128)], i4r[:, c, :],
                                     start=(c == 0), stop=(c == NDC - 1))
                nc.scalar.activation(atok[:, qt, :], t_ps[:], AF.Identity)

            std_all = stp.tile([128, 8], F32, tag="stdall")
            rstd_all = stp.tile([128, 8], F32, tag="rstdall")
            ln_tiles = []
            for qhf in range(2):
                q0 = qhf * 512
                h1 = h1p.tile([128, NFT, 512], F32R, tag="h1")
                for ft in range(NFT):
                    h_ps = ps_h.tile([128, 512], F32, tag="h")
                    for c in range(NDC):
                        nc.tensor.matmul(
                            h_ps[:],
                            W1[:, c, ts(ft, 128)],
                            AOT[:, c, q0:q0 + 512],
                            start=(c == 0), stop=(c == NDC - 1),
                        )
                    nc.scalar.activation(h1[:, ft, :], h_ps[:], AF.Relu,
                                         bias=b1[:, ft:ft + 1])
                for ql in range(4):
                    qt = qhf * 4 + ql
                    y_ps = ps_y.tile([128, D], F32, tag="y")
                    for ft in range(NFT):
                        nc.tensor.matmul(y_ps[:], h1[:, ft, ts(ql, 128)],
                                         W2[:, ft, :],
                                         start=(ft == 0), stop=(ft == NFT - 1))
                    y_sb = yp.tile([128, D], F32, tag="ysb", bufs=9)
                    nc.vector.tensor_add(y_sb[:], y_ps[:], atok[:, qt, :])
                    nc.vector.tensor_add(y_sb[:], y_sb[:], b2_bc[:])

                    stats = stp.tile([128, 6], F32, tag="st6")
                    nc.vector.bn_stats(stats[:], y_sb[:])
                    mv = stp.tile([128, 2], F32, tag="mv", bufs=9)
                    nc.vector.bn_aggr(mv[:], stats[:])
                    nc.scalar.activation(std_all[:, qt:qt + 1], mv[:, 1:2],
                                         AF.Sqrt, bias=eps_sb[:, 0:1])
                    ln_tiles.append((qt, y_sb, mv))

            nc.vector.reciprocal(rstd_all[:], std_all[:])
            for qt, y_sb, mv in ln_tiles:
                o_sb = yp.tile([128, D], F32, tag="osb")
                nc.vector.tensor_scalar(o_sb[:], y_sb[:],
                                        scalar1=mv[:, 0:1],
                                        scalar2=rstd_all[:, qt:qt + 1],
                                        op0=OP.subtract, op1=OP.mult)
                nc.vector.tensor_mul(o_sb[:], o_sb[:], g_bc[:])
                nc.vector.tensor_add(o_sb[:], o_sb[:], be_bc[:])
                nc.sync.dma_start(y_d[ts(qt, 128), :], o_sb[:])

        persist.release()

    nc.compile()
    nc.finalize()
    return nc


def _get_nc():
    global _CACHED_NC
    if _CACHED_NC is None:
        _CACHED_NC = _build()
    return _CACHED_NC


def _prep_in_maps(inputs):
    x = np.asarray(inputs["x"], dtype=np.float32)
    shared = {
        "Wq": np.ascontiguousarray(np.asarray(inputs["Wq"], np.float32)),
        "Wk": np.ascontiguousarray(np.asarray(inputs["Wk"], np.float32)),
        "Wv": np.ascontiguousarray(np.asarray(inputs["Wv"], np.float32)),
        "bqr": np.ascontiguousarray(
            np.asarray(inputs["bq"], np.float32).reshape(NDC, 128).T),
        "bkr": np.ascontiguousarray(
            np.asarray(inputs["bk"], np.float32).reshape(NDC, 128).T),
        "bv": np.ascontiguousarray(np.asarray(inputs["bv"], np.float32)),
        "W1": np.ascontiguousarray(np.asarray(inputs["W1"], np.float32)),
        "b1r": np.ascontiguousarray(
            np.asarray(inputs["b1"], np.float32).reshape(NFT, 128).T),
        "W2": np.ascontiguousarray(np.asarray(inputs["W2"], np.float32)),
        "b2": np.ascontiguousarray(np.asarray(inputs["b2"], np.float32)),
        "gamma": np.ascontiguousarray(np.asarray(inputs["gamma"], np.float32)),
        "beta": np.ascontiguousarray(np.asarray(inputs["beta"], np.float32)),
    }
    in_maps = []
    for c in range(8):
        b, qh = c // 2, c % 2
        xb = x[b]
        xp = np.concatenate([xb[qh * QB:(qh + 1) * QB],
                             xb[(1 - qh) * QB:(2 - qh) * QB]], axis=0)
        in_maps.append({**shared, "xT": np.ascontiguousarray(xp.T)})
    return in_maps


def _assemble(results):
    out = np.empty((B, S, D), np.float32)
    atn = np.empty((B, H, S, S), np.float32)
    for c in range(8):
        b, qh = c // 2, c % 2
        q0 = qh * QB
        out[b, q0:q0 + QB] = results[c]["y"]
        a = np.swapaxes(results[c]["atnT"], 1, 2)  # [H, q, k] (k in permuted order)
        if qh == 0:
            atn[b, :, q0:q0 + QB, :] = a
        else:
            atn[b, :, q0:q0 + QB, 0:QB] = a[:, :, QB:]
            atn[b, :, q0:q0 + QB, QB:] = a[:, :, 0:QB]
    return out, atn


def run(inputs, trace=False):
    nc = _get_nc()
    in_maps = _prep_in_maps(inputs)
    res = run_bass_kernel_spmd(nc, in_maps, core_ids=list(range(8)), trace=trace)
    out, atn = _assemble(res.results)
    return (out, atn), res


def kernel(**inputs):
    (out, atn), _ = run(inputs, trace=False)
    return (out, atn)
